# revision 11
# baseline (speedup 1.0000x reference)
"""ExpertBank Trainium2 kernel: LayerNorm -> per-expert [affine -> Linear(512,1024)
-> GELU(erf) -> Linear(1024,512)] for all 16 experts, expert-parallel over 8 cores.

Strategy per core (2 experts):
  - Tokens are DMA-loaded *transposed* straight from DRAM (strided access,
    512B contiguous runs), so the PE runs nothing but the 2048 GEMM matmuls --
    no on-chip transposes at all.
  - The LayerNorm is algebraically commuted through GEMM1:
      W1c.T @ ((x - mu) * rstd)  =  (W1c.T @ x - mu * colsum(W1c)) * rstd
    so GEMM1 multiplies raw (f32r) tokens, and two DVE ops per PSUM tile apply
    the rank-1 mean correction and the per-token rstd scale before the GELU.
    (mu/rstd come from host-side stats; the LN affine is folded into W1/b1.)
  - GEMM1 in fp32r (tf32-like, 1 cyc/row), GEMM2 in fp16 (1 cyc/row);
    PSUM accumulates fp32 everywhere.
  - Output [8192, 2, 512] per core; host concatenates the expert axis.
"""
import numpy as np

import concourse.tile as tile
import concourse.mybir as mybir
from concourse import bacc
from concourse.bass import ds
from concourse.bass_utils import run_bass_kernel_spmd

F32 = mybir.dt.float32
F32R = mybir.dt.float32r
FP16 = mybir.dt.float16

B, S, H, F, E = 4, 2048, 512, 1024, 16
N = B * S                 # 8192 tokens
NCORES = 8
E_LOC = E // NCORES       # 2 experts per core
EPS = 1e-5
TBLK = 1024               # tokens per block
NBLK = N // TBLK          # 8
KH = H // 128             # 4 contraction chunks for GEMM1
KF = F // 128             # 8 contraction chunks for GEMM2
MF = F // 128             # 8 output chunks for GEMM1
MT = TBLK // 128          # 8 token subtiles per block

GELU = mybir.ActivationFunctionType.Gelu
ADD = mybir.AluOpType.add
MULT = mybir.AluOpType.mult

_COMPILED = None


def _build():
    nc = bacc.Bacc("TRN2", debug=False, enable_asserts=False,
                   target_bir_lowering=False)
    tok_d = nc.dram_tensor("tokens", [N, H], F32R, kind="ExternalInput").ap()
    w1_d = nc.dram_tensor("w1", [E_LOC, KH, 128, F], F32R, kind="ExternalInput").ap()
    w2_d = nc.dram_tensor("w2", [E_LOC, KF, 128, H], FP16, kind="ExternalInput").ap()
    b1_d = nc.dram_tensor("b1c", [128, E_LOC * MF], F32, kind="ExternalInput").ap()
    b2_d = nc.dram_tensor("b2bc", [E_LOC, 128, H], F32, kind="ExternalInput").ap()
    cs_d = nc.dram_tensor("colsum", [128, E_LOC * MF], F32, kind="ExternalInput").ap()
    nmu_d = nc.dram_tensor("negmu", [128, N], FP16, kind="ExternalInput").ap()
    srw_d = nc.dram_tensor("srow", [128, N], FP16, kind="ExternalInput").ap()
    out_d = nc.dram_tensor("out", [N, E_LOC, H], F32, kind="ExternalOutput").ap()

    with tile.TileContext(nc) as tc:
        with tc.tile_pool(name="const", bufs=1) as cst, \
             tc.tile_pool(name="io", bufs=1) as io, \
             tc.tile_pool(name="ps", bufs=1, space="PSUM") as ps:

            def emit_loads(b):
                """Strided transposed token loads for block b: xTr[k][h, t]."""
                xTr = [io.tile([128, TBLK], F32R, name="xTr", tag="xTr",
                               bufs=2 * KH) for _ in range(KH)]
                for k in range(KH):
                    src = tok_d[ds(b * TBLK, TBLK),
                                ds(k * 128, 128)].transpose([1, 0])
                    nc.sync.dma_start(xTr[k], src)
                return xTr

            def emit_experts(b, xTr):
                tok0 = b * TBLK
                for e in range(E_LOC):
                    # --- GEMM1 on raw tokens + LN fixup + GELU ---
                    hT = [io.tile([128, TBLK], FP16, name="hT", tag="hT",
                                  bufs=12) for _ in range(MF)]
                    for mf in range(MF):
                        col = e * MF + mf
                        for hf in range(TBLK // 512):
                            tsl = ds(tok0 + hf * 512, 512)
                            pm1 = ps.tile([128, 512], F32, name="pm1",
                                          tag="pm1", bufs=4)
                            for k in range(KH):
                                nc.tensor.matmul(pm1,
                                                 w1t[e][k][:, ds(mf * 128, 128)],
                                                 xTr[k][:, ds(hf * 512, 512)],
                                                 start=(k == 0),
                                                 stop=(k == KH - 1))
                            # t1 = psum - mu[t]*colsum[f] ; t2 = t1 * rstd[t]
                            t1 = io.tile([128, 512], F32, name="t1", tag="t1",
                                         bufs=4)
                            nc.vector.scalar_tensor_tensor(
                                t1, nmu_t[:, tsl], cst_t[:, col:col + 1], pm1,
                                MULT, ADD)
                            t2 = io.tile([128, 512], FP16, name="t2", tag="t2",
                                         bufs=4)
                            nc.vector.tensor_tensor(t2, t1, srw_t[:, tsl], MULT)
                            nc.scalar.activation(hT[mf][:, ds(hf * 512, 512)],
                                                 t2, GELU,
                                                 bias=b1t[:, col:col + 1],
                                                 scale=1.0)
                    # --- GEMM2 + b2: out[tok, :] = hT.T @ W2c + b2 ---
                    for mt in range(MT):
                        pm2 = ps.tile([128, H], F32, name="pm2", tag="pm2",
                                      bufs=4)
                        for k in range(KF):
                            nc.tensor.matmul(pm2, hT[k][:, ds(mt * 128, 128)],
                                             w2t[e][k], start=(k == 0),
                                             stop=(k == KF - 1))
                        o_t = io.tile([128, H], F32, name="o_t", tag="o_t",
                                      bufs=6)
                        nc.vector.tensor_tensor(o_t, pm2, b2t[e], ADD)
                        nc.sync.dma_start(
                            out_d[ds(tok0 + mt * 128, 128), e, :], o_t)

            # block-0 token loads queue before the bulk constant/weight DMAs
            xTr_cur = emit_loads(0)
            cst_t = cst.tile_from(cs_d, name="cst_t")
            b1t = cst.tile_from(b1_d, name="b1t")
            b2t = [cst.tile_from(b2_d[e], name=f"b2_{e}") for e in range(E_LOC)]
            w1t = [None] * E_LOC
            w2t = [None] * E_LOC
            w1t[0] = [cst.tile_from(w1_d[0, k], name=f"w1_0_{k}")
                      for k in range(KH)]
            nmu_t = cst.tile_from(nmu_d, name="nmu_t")
            srw_t = cst.tile_from(srw_d, name="srw_t")
            w2t[0] = [cst.tile_from(w2_d[0, k], name=f"w2_0_{k}")
                      for k in range(KF)]
            w1t[1] = [cst.tile_from(w1_d[1, k], name=f"w1_1_{k}")
                      for k in range(KH)]
            w2t[1] = [cst.tile_from(w2_d[1, k], name=f"w2_1_{k}")
                      for k in range(KF)]

            for b in range(NBLK):
                xTr = xTr_cur
                if b + 1 < NBLK:
                    xTr_cur = emit_loads(b + 1)
                emit_experts(b, xTr)
    nc.compile()
    return nc


def _get_compiled():
    global _COMPILED
    if _COMPILED is None:
        _COMPILED = _build()
    return _COMPILED


def _prepare_in_maps(tokens, ln_g, ln_b, W1, b1, W2, b2):
    x = np.ascontiguousarray(np.asarray(tokens, dtype=np.float32).reshape(N, H))
    # LN stats (float64 internally; matches fp32 reference to ~1e-7 rel)
    x64 = x.astype(np.float64)
    mu = x64.mean(axis=1)
    var = np.square(x64 - mu[:, None]).mean(axis=1)
    rstd = 1.0 / np.sqrt(var + EPS)

    # Fold LN affine into W1/b1: (x_hat*g + b) @ W1 + b1 = x_hat @ (g*W1) + (b@W1 + b1)
    W1 = np.asarray(W1, dtype=np.float32)
    W2 = np.asarray(W2, dtype=np.float32)
    ln_g = np.asarray(ln_g, dtype=np.float32)
    ln_b = np.asarray(ln_b, dtype=np.float32)
    b1 = np.asarray(b1, dtype=np.float32)
    b2 = np.asarray(b2, dtype=np.float32)
    W1eff = (ln_g[:, :, None] * W1).astype(np.float32)
    b1eff = (np.einsum('eh,ehf->ef', ln_b.astype(np.float64),
                       W1.astype(np.float64)) + b1).astype(np.float32)
    colsum = W1eff.astype(np.float64).sum(axis=1).astype(np.float32)  # [E, F]
    W2h = W2.astype(np.float16)

    negmu = np.ascontiguousarray(np.broadcast_to(
        (-mu).astype(np.float16)[None, :], (128, N)))
    srow = np.ascontiguousarray(np.broadcast_to(
        rstd.astype(np.float16)[None, :], (128, N)))

    in_maps = []
    for c in range(NCORES):
        e0 = c * E_LOC
        sl = slice(e0, e0 + E_LOC)
        in_maps.append({
            "tokens": x,
            "w1": np.ascontiguousarray(W1eff[sl].reshape(E_LOC, KH, 128, F)),
            "w2": np.ascontiguousarray(W2h[sl].reshape(E_LOC, KF, 128, H)),
            "b1c": np.ascontiguousarray(
                b1eff[sl].reshape(E_LOC, MF, 128).transpose(2, 0, 1)
                .reshape(128, E_LOC * MF)),
            "colsum": np.ascontiguousarray(
                colsum[sl].reshape(E_LOC, MF, 128).transpose(2, 0, 1)
                .reshape(128, E_LOC * MF)),
            "b2bc": np.ascontiguousarray(
                np.broadcast_to(b2[sl][:, None, :], (E_LOC, 128, H))),
            "negmu": negmu,
            "srow": srow,
        })
    return in_maps


def _run(in_maps, trace=False, **kw):
    nc = _get_compiled()
    return run_bass_kernel_spmd(nc, in_maps, core_ids=list(range(NCORES)),
                                trace=trace, **kw)


def kernel(tokens, ln_g, ln_b, W1, b1, W2, b2):
    in_maps = _prepare_in_maps(tokens, ln_g, ln_b, W1, b1, W2, b2)
    res = _run(in_maps)
    parts = [res.results[c]["out"] for c in range(NCORES)]   # [N, E_LOC, H] each
    full = np.concatenate(parts, axis=1).reshape(B, S, E, H)
    return full.astype(np.float32)
